# revision 27
# baseline (speedup 1.0000x reference)
"""Slot-attention corrector kernel for Trainium2 (8 NeuronCores, data-parallel).

Design (v2, pipelined; TimelineSim 309us vs 611us for the v1 baseline):
  - dual input streams per example, batched quarter-DMAs (1024 tokens):
    x natural in fp8e4m3 (feeds bn_stats only; noise averages out over F=512)
    and xT in bf16 (feeds the k/v matmuls)
  - LN folded into k/v: gamma into wkv; mean correction as rank-1 K=1 matmuls
    from a [1,1024] -mu row (PE transpose + DRAM-bounce flatten per quarter)
  - kT [d,n] and v [n,d] resident bf16, both UNSCALED by rstd:
    k-side rstd is applied to the dots in PSUM (DVE mul, free-broadcast view),
    v-side rstd rides the attention weights (attn_u = E * rden * rstd) and the
    z-denominator uses std-column lhsT matmuls (z = sum std*attn_u)
  - dots^T [n-part, (t,s)] so softmax over slots is a free-axis group reduce;
    updates flipped (lhsT=v block, rhs=attn 16-col) -> updT [d,s] directly,
    16-wide PE streams (output-free-size is what the PE engine pays for)
  - iter-1 attention software-pipelined into the phase-1 example loop
    (front: dots..attn with 1-example lag; back: updates with 2-example lag);
    iters 2+ run all fronts then all backs so engines pipeline across examples
  - Act engine pinned to ONE table set (natural_log_exp_and_others) for the
    whole kernel: rstd/std = exp(-/+0.5*ln(var+eps)), sigmoid/tanh built from
    exp + reciprocal; a monkeypatch forces the table-load inserter's choice
    (it otherwise thrashes 76 LoadActFuncSet = ~97us)
  - a few stats tiles per example computed via Act Square/Copy accum_out to
    offload the DVE (bn_stats is the phase-1 pacer)
  - GRU/MLP matmuls in bf16 (fp32 PSUM accumulation, fp32 slot state)
"""

import numpy as np
import ml_dtypes
import sys

sys.path.insert(0, "/opt/trn_rl_repo")

NUM_SLOTS, SLOT_DIM, FEAT_DIM, HID_DIM = 16, 128, 512, 512
EPS_LN = 1e-3
SCALE = FEAT_DIM ** -0.5
B, N = 64, 4096
NCORES = 8
BEX = B // NCORES          # 8 examples per core
NBLK = N // 128            # 32 n-blocks per example
NQ = 4                     # quarters per example (1024 tokens each)

_CACHE = {}
X_FP8 = True    # ship the stats stream (natural x) in fp8 e4m3: mean/var
                # average over 512 elements so the ~3% element noise lands
                # ~0.2% on rstd; halves that stream's HBM traffic


def _pin_act_table():
    """Force the act-table-load inserter to use natural_log_exp_and_others
    (covers exp/ln/copy/identity/relu — every function this kernel uses) so
    exactly one table load is emitted instead of thrashing between minimal
    sets. Order is preserved so act_func_set_id stays aligned with
    act_info.json."""
    import concourse.bacc as bacc

    if getattr(bacc, "_act_tables_pinned", False):
        return
    orig = bacc.get_activation_tables

    def pinned(arch):
        tables = orig(arch)
        if "natural_log_exp_and_others" not in tables:
            return tables
        return {
            name: (funcs if name == "natural_log_exp_and_others" else set())
            for name, funcs in tables.items()
        }

    bacc.get_activation_tables = pinned
    bacc._act_tables_pinned = True


def _build(num_iters: int, general_bias: bool, reps: int = 1):
    import concourse.bass as bass
    import concourse.bacc as bacc
    import concourse.tile as tile
    from concourse import mybir

    _pin_act_table()

    f32 = mybir.dt.float32
    bf16 = mybir.dt.bfloat16
    AF = mybir.ActivationFunctionType
    AX = mybir.AxisListType
    OP = mybir.AluOpType

    nc = bacc.Bacc('TRN2', target_bir_lowering=False, debug=False, enable_asserts=False, num_devices=NCORES)

    # ---------------- dram I/O ----------------
    xdt = mybir.dt.float8e4 if X_FP8 else bf16
    x_d = nc.dram_tensor("x", [BEX, N, FEAT_DIM], xdt, kind="ExternalInput")
    xT_d = nc.dram_tensor("xT", [BEX, FEAT_DIM, N], bf16, kind="ExternalInput")
    slots_d = nc.dram_tensor("slots0", [128, SLOT_DIM], f32, kind="ExternalInput")
    wkv_d = nc.dram_tensor("wkv", [FEAT_DIM, 256], bf16, kind="ExternalInput")
    ckv_d = nc.dram_tensor("ckv", [1, 256], bf16, kind="ExternalInput")
    wq_d = nc.dram_tensor("wq", [SLOT_DIM, SLOT_DIM], bf16, kind="ExternalInput")
    bqs_col_d = nc.dram_tensor("bqs_col", [128, 1], f32, kind="ExternalInput")
    wih_d = nc.dram_tensor("wihT", [SLOT_DIM, 3 * SLOT_DIM], bf16, kind="ExternalInput")
    whh_d = nc.dram_tensor("whhT", [SLOT_DIM, 3 * SLOT_DIM], bf16, kind="ExternalInput")
    bih_d = nc.dram_tensor("bih_row", [1, 3 * SLOT_DIM], bf16, kind="ExternalInput")
    bhh_d = nc.dram_tensor("bhh_row", [1, 3 * SLOT_DIM], bf16, kind="ExternalInput")
    w1_d = nc.dram_tensor("w1", [SLOT_DIM, HID_DIM], bf16, kind="ExternalInput")
    b1c_d = nc.dram_tensor("b1_cols", [128, 4], f32, kind="ExternalInput")
    w2_d = nc.dram_tensor("w2", [HID_DIM, SLOT_DIM], bf16, kind="ExternalInput")
    b2_d = nc.dram_tensor("b2_row", [1, SLOT_DIM], bf16, kind="ExternalInput")
    ones_b_d = nc.dram_tensor("ones_b", [128, 128], bf16, kind="ExternalInput")
    ones_fr_d = nc.dram_tensor("ones_f_row", [1, 128], f32, kind="ExternalInput")
    ident_d = nc.dram_tensor("ident", [128, 128], f32, kind="ExternalInput")
    nident_d = nc.dram_tensor("nident", [128, 128], f32, kind="ExternalInput")
    if general_bias:
        bkv_d = nc.dram_tensor("bkv", [1, 256], f32, kind="ExternalInput")
        bk_colb_d = nc.dram_tensor("bk_col_b", [128, 1], bf16, kind="ExternalInput")
    out_d = nc.dram_tensor("out", [128, SLOT_DIM], f32, kind="ExternalOutput")

    from contextlib import ExitStack

    with tile.TileContext(nc) as tc:
        with ExitStack() as stack:
            pool = lambda *a, **k: stack.enter_context(tc.tile_pool(*a, **k))
            kvp = pool(name="kv", bufs=1)            # resident k/v (16MB)
            cp = pool(name="consts", bufs=1)
            srp = pool(name="statr", bufs=1)         # resident per-ex rstd
            xp = pool(name="xq", bufs=3)
            xtp = pool(name="xtq", bufs=2)
            stp = pool(name="stq", bufs=2)
            nmp = pool(name="nmup", bufs=2)
            atp = pool(name="attnS", bufs=8)         # per-ex attn tiles
            edp = pool(name="eds", bufs=2)
            dnp = pool(name="denb", bufs=3)
            qbp = pool(name="qb", bufs=2)
            gp = pool(name="gru", bufs=1)
            slp = pool(name="slt", bufs=2)
            pa = pool(name="pbig", bufs=2, space="PSUM")     # kps/dps [128,512] f32, 2 banks
            pb = pool(name="pv", bufs=2, space="PSUM")       # v 4-block banks, 2 banks
            pc = pool(name="psc", bufs=2, space="PSUM")      # shared scratch banks, 2 banks
            pd = pool(name="pgate", bufs=1, space="PSUM")    # gi/gh [128,384], 2 banks
            dp = pool(name="dram", bufs=3, space="DRAM")
            p1ps = pa
            pdots = pa
            pgate = pd
            # ---- resident k/v ----
            kT = [kvp.tile([128, N], bf16, tag=f"kT{e}", name=f"kT{e}") for e in range(BEX)]
            vN = [kvp.tile([128, N], bf16, tag=f"v{e}", name=f"v{e}") for e in range(BEX)]
            rstd_nat = [srp.tile([128, NBLK], f32, tag=f"rstd{e}", name=f"rstd{e}") for e in range(BEX)]
            std_b = [srp.tile([128, NBLK], bf16, tag=f"std{e}", name=f"std{e}") for e in range(BEX)]

            # ---- constants ----
            wkv_sb = cp.tile([FEAT_DIM // 4, 4, 256], bf16)  # [128f, fch, 256]
            for j in range(4):
                nc.sync.dma_start(out=wkv_sb[:, j, :], in_=wkv_d[j * 128:(j + 1) * 128, :])
            ckv_sb = cp.tile([1, 256], bf16)
            nc.sync.dma_start(out=ckv_sb, in_=ckv_d[:, :])
            wq_sb = cp.tile([128, 128], bf16)
            nc.sync.dma_start(out=wq_sb, in_=wq_d[:, :])
            bqs_sb = cp.tile([128, 1], f32)
            nc.sync.dma_start(out=bqs_sb, in_=bqs_col_d[:, :])
            wih_sb = cp.tile([128, 384], bf16)
            nc.sync.dma_start(out=wih_sb, in_=wih_d[:, :])
            whh_sb = cp.tile([128, 384], bf16)
            nc.sync.dma_start(out=whh_sb, in_=whh_d[:, :])
            bih_sb = cp.tile([1, 384], bf16)
            nc.sync.dma_start(out=bih_sb, in_=bih_d[:, :])
            bhh_sb = cp.tile([1, 384], bf16)
            nc.sync.dma_start(out=bhh_sb, in_=bhh_d[:, :])
            w1_sb = cp.tile([128, 512], bf16)
            nc.sync.dma_start(out=w1_sb, in_=w1_d[:, :])
            b1c_sb = cp.tile([128, 4], f32)
            nc.sync.dma_start(out=b1c_sb, in_=b1c_d[:, :])
            w2_sb = cp.tile([128, 4, 128], bf16)  # [128h, chunk, 128d]
            for j in range(4):
                nc.sync.dma_start(out=w2_sb[:, j, :], in_=w2_d[j * 128:(j + 1) * 128, :])
            b2_sb = cp.tile([1, 128], bf16)
            nc.sync.dma_start(out=b2_sb, in_=b2_d[:, :])
            ones_b = cp.tile([128, 128], bf16)
            nc.sync.dma_start(out=ones_b, in_=ones_b_d[:, :])
            ones_fr = cp.tile([1, 128], f32)
            nc.sync.dma_start(out=ones_fr, in_=ones_fr_d[:, :])
            ident = cp.tile([128, 128], f32)
            nc.sync.dma_start(out=ident, in_=ident_d[:, :])
            nident = cp.tile([128, 128], f32)
            nc.sync.dma_start(out=nident, in_=nident_d[:, :])
            eps_col = cp.tile([128, 1], f32)
            nc.vector.memset(eps_col, EPS_LN)
            neghalf_col = cp.tile([128, 1], f32)
            nc.vector.memset(neghalf_col, -0.5)
            poshalf_col = cp.tile([128, 1], f32)
            nc.vector.memset(poshalf_col, 0.5)
            negone_col = cp.tile([128, 1], f32)
            nc.vector.memset(negone_col, -1.0)
            negtwo_col = cp.tile([128, 1], f32)
            nc.vector.memset(negtwo_col, -2.0)
            if general_bias:
                bk_colb = cp.tile([128, 1], bf16)
                nc.sync.dma_start(out=bk_colb, in_=bk_colb_d[:, :])
                bv_bc = cp.tile([128, 128], f32)
                nc.sync.dma_start(
                    out=bv_bc,
                    in_=bass.AP(tensor=bkv_d, offset=128, ap=[[0, 128], [1, 128]]),
                )

            def layernorm_T(src, tag):
                """LN over free dim of [128,128] f32 src -> lnT bf16 [128,128]."""
                st = gp.tile([128, 6], f32, tag=f"{tag}_st")
                nc.vector.bn_stats(out=st, in_=src)
                mv = gp.tile([128, 2], f32, tag=f"{tag}_mv")
                nc.vector.bn_aggr(out=mv, in_=st)
                lnv = gp.tile([128, 1], f32, tag=f"{tag}_lnv")
                nc.scalar.activation(lnv, mv[:, 1:2], AF.Ln, bias=eps_col)
                rstd = gp.tile([128, 1], f32, tag=f"{tag}_rstd")
                nc.scalar.activation(rstd, lnv, AF.Exp, scale=neghalf_col)
                nmr = gp.tile([128, 1], f32, tag=f"{tag}_nmr")
                nc.vector.scalar_tensor_tensor(nmr, mv[:, 0:1], -1.0, rstd, OP.mult, OP.mult)
                ln = gp.tile([128, 128], f32, tag=f"{tag}_ln")
                nc.scalar.activation(ln, src, AF.Identity, scale=rstd, bias=nmr)
                sc = pc.tile([128, 512], f32, tag="sc")
                nc.tensor.transpose(sc[:, 0:128], ln, ident)
                lnT = gp.tile([128, 128], bf16, tag=f"{tag}_lnT")
                nc.scalar.activation(lnT, sc[:, 0:128], AF.Copy)
                return lnT, sc

            def qpath(slots):
                lnT, sc = layernorm_T(slots, "q")
                qps = sc[:, 128:256]
                nc.tensor.matmul(qps, wq_sb, lnT)
                qT = qbp.tile([128, 128], bf16, tag="qT")
                nc.scalar.activation(qT, qps, AF.Identity, bias=bqs_sb)
                qbk = None
                if general_bias:
                    qbk_ps = sc[0:1, 256:384]
                    nc.tensor.matmul(qbk_ps, bk_colb, qT)
                    qbk = qbp.tile([1, 128], bf16, tag="qbk")
                    nc.scalar.activation(qbk, qbk_ps, AF.Copy)
                return qT, qbk

            def phase1(e):
                for qd in range(NQ):
                    xq = xp.tile([128, 8, 512], xdt, tag="xq")
                    nc.sync.dma_start(
                        out=xq,
                        in_=bass.AP(tensor=x_d, offset=(e * N + qd * 1024) * FEAT_DIM,
                                    ap=[[FEAT_DIM, 128], [128 * FEAT_DIM, 8], [1, FEAT_DIM]]),
                    )
                    xtq = xtp.tile([128, 4, 1024], bf16, tag="xtq")
                    nc.sync.dma_start(
                        out=xtq,
                        in_=bass.AP(tensor=xT_d, offset=e * FEAT_DIM * N + qd * 1024,
                                    ap=[[N, 128], [128 * N, 4], [1, 1024]]),
                    )
                    st = stp.tile([128, 8, 6], f32, tag="st")
                    mvq = stp.tile([128, 8, 2], f32, tag="mv")
                    for t8 in range(8):
                        if qd < 3 and t8 == 0:
                            # Act-accumulated stats: sum(x^2) and sum(x)
                            sq = stp.tile([128, 512], bf16, tag="sqscrap")
                            s2c = stp.tile([128, 1], f32, tag="s2c")
                            nc.scalar.activation(sq, xq[:, t8, :], AF.Square,
                                                 accum_out=s2c)
                            cpx = stp.tile([128, 512], bf16, tag="cpscrap")
                            s1c = stp.tile([128, 1], f32, tag="s1c")
                            nc.scalar.activation(cpx, xq[:, t8, :], AF.Copy,
                                                 accum_out=s1c)
                            nc.vector.tensor_scalar_mul(mvq[:, t8, 0:1], s1c,
                                                        1.0 / FEAT_DIM)
                            msq = stp.tile([128, 1], f32, tag="msq")
                            nc.vector.tensor_mul(msq, mvq[:, t8, 0:1], mvq[:, t8, 0:1])
                            nc.vector.scalar_tensor_tensor(
                                mvq[:, t8, 1:2], s2c, 1.0 / FEAT_DIM, msq,
                                OP.mult, OP.subtract)
                        else:
                            nc.vector.bn_stats(out=st[:, t8, :], in_=xq[:, t8, :])
                            nc.vector.bn_aggr(out=mvq[:, t8, :], in_=st[:, t8, :])
                    # rstd = exp(-0.5 * ln(var + eps))  (keeps Act on the exp/ln table)
                    lnv8 = stp.tile([128, 8], f32, tag="lnv8")
                    nc.scalar.activation(lnv8, mvq[:, :, 1], AF.Ln, bias=eps_col)
                    nc.scalar.activation(rstd_nat[e][:, 8 * qd:8 * qd + 8], lnv8,
                                         AF.Exp, scale=neghalf_col)
                    nc.scalar.activation(std_b[e][:, 8 * qd:8 * qd + 8], lnv8,
                                         AF.Exp, scale=poshalf_col)
                    # -mu, transposed then flattened to a [1,1024] row
                    # (matmul lhsT/rhs must sit at base partition 0)
                    sc1 = pc.tile([128, 512], f32, tag="sc")
                    nc.tensor.transpose(sc1[0:8, 0:128], mvq[:, :, 0], nident)
                    nmuT_sb = nmp.tile([8, 128], bf16, tag="nmuT")
                    nc.scalar.activation(nmuT_sb, sc1[0:8, 0:128], AF.Copy)
                    dr = dp.tile([8, 128], bf16, tag="bounce")
                    nc.sync.dma_start(out=dr, in_=nmuT_sb)
                    nmu_row = nmp.tile([1, 1024], bf16, tag="nmurow")
                    nc.sync.dma_start(
                        out=nmu_row,
                        in_=bass.AP(tensor=dr.tensor, offset=dr.offset, ap=[[0, 1], [1, 1024]]),
                    )

                    # ---- kT chunks (2 per quarter), unscaled ----
                    for ci in range(2):
                        c = 2 * qd + ci
                        ps = p1ps.tile([128, 512], f32, tag="big512")
                        for j in range(4):
                            nc.tensor.matmul(ps, wkv_sb[:, j, 0:128],
                                             xtq[:, j, ci * 512:(ci + 1) * 512],
                                             start=(j == 0), stop=False)
                        for m4 in range(4):
                            loc = 4 * ci + m4      # tile index within quarter
                            nc.tensor.matmul(ps[:, m4 * 128:(m4 + 1) * 128],
                                             ckv_sb[:, 0:128],
                                             nmu_row[0:1, loc * 128:(loc + 1) * 128],
                                             start=False, stop=(m4 == 3),
                                             skip_group_check=True)
                        nc.scalar.activation(kT[e][:, c * 512:(c + 1) * 512], ps, AF.Copy)

                    # ---- v blocks (8 per quarter), rstd folded at evacuation ----
                    for tb in range(8):
                        t = 8 * qd + tb
                        if tb % 4 == 0:
                            vbank = pb.tile([128, 4, 128], f32, tag="vps")
                        vps = vbank[:, tb % 4, :]
                        for j in range(4):
                            nc.tensor.matmul(vps, xtq[:, j, tb * 128:(tb + 1) * 128],
                                             wkv_sb[:, j, 128:256],
                                             start=(j == 0), stop=False)
                        nc.tensor.matmul(vps, nmu_row[0:1, tb * 128:(tb + 1) * 128],
                                         ckv_sb[:, 128:256],
                                         start=False, stop=True)
                        if general_bias:
                            # bias must not be rstd-scaled; v stays unscaled here
                            # (rstd folds into attn), so scale bv by std at add
                            vs = stp.tile([128, 128], f32, tag="vtmp")
                            nc.scalar.activation(vs, bv_bc, AF.Copy,
                                                 scale=std_b[e][:, t:t + 1])
                            nc.vector.tensor_add(vps, vps, vs)
                        if tb % 4 == 3:
                            nc.scalar.activation(
                                vN[e][:, (t - 3) * 128:(t + 1) * 128], vbank, AF.Copy)

            attn_sb = [None] * BEX

            def attn_front(e, qT, qbk):
                dps = pdots.tile([128, 512], f32, tag="big512")
                for t in range(NBLK):
                    if general_bias:
                        nc.tensor.matmul(dps[:, t * 16:(t + 1) * 16],
                                         kT[e][:, t * 128:(t + 1) * 128],
                                         qT[:, e * 16:(e + 1) * 16],
                                         start=True, stop=False)
                        nc.tensor.matmul(dps[:, t * 16:(t + 1) * 16],
                                         ones_b[0:1, :], qbk[:, e * 16:(e + 1) * 16],
                                         start=False, stop=True)
                    else:
                        nc.tensor.matmul(dps[:, t * 16:(t + 1) * 16],
                                         kT[e][:, t * 128:(t + 1) * 128],
                                         qT[:, e * 16:(e + 1) * 16])
                # k-side rstd applied on the dots (free-broadcast over s)
                ds = edp.tile([128, 512], f32, tag="ds")
                nc.vector.tensor_mul(
                    ds, dps,
                    bass.AP(tensor=rstd_nat[e].tensor, offset=rstd_nat[e].offset,
                            ap=[rstd_nat[e].ap[0], [1, NBLK], [0, 16]]),
                )
                E = edp.tile([128, 512], bf16, tag="E")
                nc.scalar.activation(E, ds, AF.Exp)
                den = dnp.tile([128, NBLK], bf16, tag="den")
                with nc.allow_low_precision(reason="softmax denominator, 16 terms"):
                    nc.vector.reduce_sum(
                        den, bass.AP(tensor=E.tensor, offset=E.offset,
                                     ap=[E.ap[0], [16, NBLK], [1, 16]]),
                        axis=AX.X,
                    )
                rden = dnp.tile([128, NBLK], f32, tag="rden")
                nc.vector.reciprocal(rden, den)
                rr = dnp.tile([128, NBLK], f32, tag="rr")
                nc.vector.tensor_mul(rr, rden, rstd_nat[e])
                attn = atp.tile([128, 512], bf16, tag="attn")
                nc.gpsimd.tensor_mul(
                    attn, E,
                    bass.AP(tensor=rr.tensor, offset=rr.offset,
                            ap=[rr.ap[0], [1, NBLK], [0, 16]]),
                )
                attn_sb[e] = attn

            def attn_back(e, updT):
                attn = attn_sb[e]
                sc = pc.tile([128, 512], f32, tag="sc")
                ups = sc[:, 0:16]
                zps = sc[0:1, 16:32]
                zbc = sc[:, 32:48]
                for t in range(NBLK):
                    nc.tensor.matmul(ups, vN[e][:, t * 128:(t + 1) * 128],
                                     attn[:, t * 16:(t + 1) * 16],
                                     start=(t == 0), stop=(t == NBLK - 1))
                for t in range(NBLK):
                    nc.tensor.matmul(zps, std_b[e][:, t:t + 1],
                                     attn[:, t * 16:(t + 1) * 16],
                                     start=(t == 0), stop=(t == NBLK - 1))
                rz = dnp.tile([1, 16], f32, tag="rz")
                nc.vector.reciprocal(rz, zps)
                nc.tensor.matmul(zbc, ones_fr, rz)
                zbs = dnp.tile([128, 16], f32, tag="zbs")
                nc.scalar.activation(zbs, zbc, AF.Copy)
                nc.vector.tensor_mul(updT[:, e * 16:(e + 1) * 16], ups, zbs)

            def sigmoid_via_exp(out, in_ap, width, tag):
                e1 = gp.tile([128, width], f32, tag=f"{tag}_e")
                nc.scalar.activation(e1, in_ap, AF.Exp, scale=negone_col)
                p1 = gp.tile([128, width], f32, tag=f"{tag}_p")
                nc.vector.tensor_scalar_add(p1, e1, 1.0)
                nc.vector.reciprocal(out, p1)

            def grumlp(slots, updT):
                gips = pgate.tile([128, 384], f32, tag="gi")
                nc.tensor.matmul(gips, updT, wih_sb, start=True, stop=False)
                nc.tensor.matmul(gips, ones_b[0:1, :], bih_sb, start=False, stop=True)
                scg = pc.tile([128, 512], f32, tag="sc")
                nc.tensor.transpose(scg[:, 0:128], slots, ident)
                slotsT = gp.tile([128, 128], bf16, tag="slotsT")
                nc.scalar.activation(slotsT, scg[:, 0:128], AF.Copy)
                ghps = pgate.tile([128, 384], f32, tag="gh")
                nc.tensor.matmul(ghps, slotsT, whh_sb, start=True, stop=False)
                nc.tensor.matmul(ghps, ones_b[0:1, :], bhh_sb, start=False, stop=True)
                gh_sb = gp.tile([128, 384], f32, tag="ghsb")
                nc.scalar.activation(gh_sb, ghps, AF.Copy)
                rzin = gp.tile([128, 256], f32, tag="rzin")
                nc.vector.tensor_add(rzin, gips[:, 0:256], gh_sb[:, 0:256])
                rzg = gp.tile([128, 256], f32, tag="rzg")
                sigmoid_via_exp(rzg, rzin, 256, "sg")
                hnr = gp.tile([128, 128], f32, tag="hnr")
                nc.vector.tensor_mul(hnr, rzg[:, 0:128], gh_sb[:, 256:384])
                nin = gp.tile([128, 128], f32, tag="nin")
                nc.vector.tensor_add(nin, gips[:, 256:384], hnr)
                # tanh(x) = 2/(1+exp(-2x)) - 1
                e2 = gp.tile([128, 128], f32, tag="th_e")
                nc.scalar.activation(e2, nin, AF.Exp, scale=negtwo_col)
                p2 = gp.tile([128, 128], f32, tag="th_p")
                nc.vector.tensor_scalar_add(p2, e2, 1.0)
                r2 = gp.tile([128, 128], f32, tag="th_r")
                nc.vector.reciprocal(r2, p2)
                ng = gp.tile([128, 128], f32, tag="ng")
                nc.vector.tensor_scalar(ng, r2, 2.0, -1.0, OP.mult, OP.add)
                hmn = gp.tile([128, 128], f32, tag="hmn")
                nc.vector.tensor_sub(hmn, slots, ng)
                zh = gp.tile([128, 128], f32, tag="zh")
                nc.vector.tensor_mul(zh, rzg[:, 128:256], hmn)
                hgru = gp.tile([128, 128], f32, tag="hgru")
                nc.vector.tensor_add(hgru, ng, zh)

                lnmT, scm = layernorm_T(hgru, "m")
                h1r = gp.tile([128, 4, 128], bf16, tag="h1r")
                sch = pc.tile([128, 512], f32, tag="sc")
                for j in range(4):
                    hp = sch[:, j * 128:(j + 1) * 128]
                    nc.tensor.matmul(hp, w1_sb[:, j * 128:(j + 1) * 128], lnmT)
                    nc.scalar.activation(h1r[:, j, :], hp, AF.Relu, bias=b1c_sb[:, j:j + 1])
                h2ps = scm[:, 128:256]
                for j in range(4):
                    nc.tensor.matmul(h2ps, h1r[:, j, :], w2_sb[:, j, :],
                                     start=(j == 0), stop=False)
                nc.tensor.matmul(h2ps, ones_b[0:1, :], b2_sb, start=False, stop=True)
                new_slots = slp.tile([128, 128], f32, tag="slots")
                nc.vector.tensor_add(new_slots, h2ps, hgru)
                return new_slots

            for _rep in range(reps):
                slots = slp.tile([128, 128], f32, tag="slots")
                nc.sync.dma_start(out=slots, in_=slots_d[:, :])

                # iter-1 q depends only on initial slots: compute up front
                qT1, qbk1 = qpath(slots)
                updT1 = qbp.tile([128, 128], bf16, tag="updT")
                for e in range(BEX):
                    if e >= 1:
                        attn_front(e - 1, qT1, qbk1)
                    phase1(e)
                    if e >= 2:
                        attn_back(e - 2, updT1)
                attn_front(BEX - 1, qT1, qbk1)
                attn_back(BEX - 2, updT1)
                attn_back(BEX - 1, updT1)
                slots = grumlp(slots, updT1)

                for _it in range(1, num_iters):
                    qT, qbk = qpath(slots)
                    updT = qbp.tile([128, 128], bf16, tag="updT")
                    for e in range(BEX):
                        attn_front(e, qT, qbk)
                    for e in range(BEX):
                        attn_back(e, updT)
                    slots = grumlp(slots, updT)

                nc.sync.dma_start(out=out_d[:, :], in_=slots)

    nc.finalize()
    return nc


def _prep_host(inputs):
    f = np.float32
    bf = ml_dtypes.bfloat16
    g_in = inputs["ln_in_g"].astype(f)
    b_in = inputs["ln_in_b"].astype(f)
    Wk = inputs["Wk"].astype(f)
    Wv = inputs["Wv"].astype(f)
    Wkp = g_in[:, None] * Wk
    Wvp = g_in[:, None] * Wv
    wkv = np.concatenate([Wkp, Wvp], axis=1)                      # [512, 256]
    ckv = wkv.sum(axis=0, keepdims=True)                          # [1, 256]
    bk = b_in @ Wk + inputs["bk"].astype(f)
    bv = b_in @ Wv + inputs["bv"].astype(f)
    bkv = np.concatenate([bk, bv])[None, :]                       # [1, 256]
    g_s = inputs["ln_slot_g"].astype(f)
    b_s = inputs["ln_slot_b"].astype(f)
    Wq = inputs["Wq"].astype(f)
    wqp = g_s[:, None] * Wq
    bqs = (b_s @ Wq + inputs["bq"].astype(f)) * np.float32(SCALE)
    g_m = inputs["ln_mlp_g"].astype(f)
    b_m = inputs["ln_mlp_b"].astype(f)
    W1 = inputs["W1"].astype(f)
    w1p = g_m[:, None] * W1
    b1p = b_m @ W1 + inputs["b1"].astype(f)                       # [512]
    consts = dict(
        wkv=wkv.astype(bf),
        ckv=ckv.astype(bf),
        wq=(wqp * np.float32(SCALE)).astype(bf),
        bqs_col=bqs[:, None].astype(f),
        wihT=np.ascontiguousarray(inputs["W_ih"].astype(f).T).astype(bf),
        whhT=np.ascontiguousarray(inputs["W_hh"].astype(f).T).astype(bf),
        bih_row=inputs["b_ih"].astype(f)[None, :].astype(bf),
        bhh_row=inputs["b_hh"].astype(f)[None, :].astype(bf),
        w1=w1p.astype(bf),
        b1_cols=np.ascontiguousarray(b1p.reshape(4, 128).T).astype(f),
        w2=inputs["W2"].astype(f).astype(bf),
        b2_row=inputs["b2"].astype(f)[None, :].astype(bf),
        ones_b=np.ones((128, 128), bf),
        ones_f_row=np.ones((1, 128), f),
        ident=np.eye(128, dtype=f),
        nident=(-np.eye(128)).astype(f),
    )
    general_bias = not (
        np.all(b_in == 0) and np.all(inputs["bk"] == 0) and np.all(inputs["bv"] == 0)
    )
    if general_bias:
        consts["bkv"] = bkv.astype(f)
        consts["bk_col_b"] = bk[:, None].astype(bf)
    return consts, general_bias


LAST_RESULT = None


def kernel(**inputs) -> np.ndarray:
    import os
    from concourse.bass_utils import run_bass_kernel_spmd

    is_first = int(np.asarray(inputs["is_first"]))
    num_iters = 3 if is_first else 2
    consts, general_bias = _prep_host(inputs)

    key = (num_iters, general_bias)
    if key not in _CACHE:
        _CACHE[key] = _build(num_iters, general_bias)
    nc = _CACHE[key]

    bf = ml_dtypes.bfloat16
    x = inputs["image_features"].astype(np.float32)
    xb = x.astype(ml_dtypes.float8_e4m3 if X_FP8 else bf)         # [64, 4096, 512]
    xTb = np.ascontiguousarray(x.transpose(0, 2, 1)).astype(bf)   # [64, 512, 4096]
    slots = inputs["slots"].astype(np.float32)                    # [64, 16, 128]

    in_maps = []
    for c in range(NCORES):
        sl = slice(c * BEX, (c + 1) * BEX)
        m = dict(consts)
        m["x"] = xb[sl]
        m["xT"] = xTb[sl]
        m["slots0"] = slots[sl].reshape(128, SLOT_DIM)
        in_maps.append(m)

    trace = bool(int(os.environ.get("KERNEL_TRACE", "0")))
    res = run_bass_kernel_spmd(nc, in_maps, list(range(NCORES)), trace=trace)
    global LAST_RESULT
    LAST_RESULT = res
    out = np.stack([res.results[c]["out"] for c in range(NCORES)])  # [8, 128, 128]
    return out.reshape(B, NUM_SLOTS, SLOT_DIM)


if __name__ == "__main__":
    import reference
    inp = reference.setup_inputs()
    inp = {k: np.asarray(v) for k, v in inp.items()}
    got = kernel(**inp)
    exp = np.asarray(reference.reference(**reference.setup_inputs()))
    err = np.linalg.norm(got - exp) / np.linalg.norm(exp)
    print("Relative error:", err)


# revision 30
# speedup vs baseline: 1.0108x; 1.0108x over previous
"""Slot-attention corrector kernel for Trainium2 (8 NeuronCores, data-parallel).

Design (v2, pipelined; TimelineSim 309us vs 611us for the v1 baseline):
  - dual input streams per example, batched quarter-DMAs (1024 tokens):
    x natural in fp8e4m3 (feeds bn_stats only; noise averages out over F=512)
    and xT in bf16 (feeds the k/v matmuls)
  - LN folded into k/v: gamma into wkv; mean correction as rank-1 K=1 matmuls
    from a [1,1024] -mu row (PE transpose + DRAM-bounce flatten per quarter)
  - kT [d,n] and v [n,d] resident bf16, both UNSCALED by rstd:
    k-side rstd is applied to the dots in PSUM (DVE mul, free-broadcast view),
    v-side rstd rides the attention weights (attn_u = E * rden * rstd) and the
    z-denominator uses std-column lhsT matmuls (z = sum std*attn_u)
  - dots^T [n-part, (t,s)] so softmax over slots is a free-axis group reduce;
    updates flipped (lhsT=v block, rhs=attn 16-col) -> updT [d,s] directly,
    16-wide PE streams (output-free-size is what the PE engine pays for)
  - iter-1 attention software-pipelined into the phase-1 example loop
    (front: dots..attn with 1-example lag; back: updates with 2-example lag);
    iters 2+ run all fronts then all backs so engines pipeline across examples
  - Act engine pinned to ONE table set (natural_log_exp_and_others) for the
    whole kernel: rstd/std = exp(-/+0.5*ln(var+eps)), sigmoid/tanh built from
    exp + reciprocal; a monkeypatch forces the table-load inserter's choice
    (it otherwise thrashes 76 LoadActFuncSet = ~97us)
  - a few stats tiles per example computed via Act Square/Copy accum_out to
    offload the DVE (bn_stats is the phase-1 pacer)
  - GRU/MLP matmuls in bf16 (fp32 PSUM accumulation, fp32 slot state)
"""

import numpy as np
import ml_dtypes
import sys

sys.path.insert(0, "/opt/trn_rl_repo")

NUM_SLOTS, SLOT_DIM, FEAT_DIM, HID_DIM = 16, 128, 512, 512
EPS_LN = 1e-3
SCALE = FEAT_DIM ** -0.5
B, N = 64, 4096
NCORES = 8
BEX = B // NCORES          # 8 examples per core
NBLK = N // 128            # 32 n-blocks per example
NQ = 4                     # quarters per example (1024 tokens each)

_CACHE = {}
X_FP8 = True    # ship the stats stream (natural x) in fp8 e4m3: mean/var
                # average over 512 elements so the ~3% element noise lands
                # ~0.2% on rstd; halves that stream's HBM traffic


def _pin_act_table():
    """Force the act-table-load inserter to use natural_log_exp_and_others
    (covers exp/ln/copy/identity/relu — every function this kernel uses) so
    exactly one table load is emitted instead of thrashing between minimal
    sets. Order is preserved so act_func_set_id stays aligned with
    act_info.json."""
    import concourse.bacc as bacc

    if getattr(bacc, "_act_tables_pinned", False):
        return
    orig = bacc.get_activation_tables

    def pinned(arch):
        tables = orig(arch)
        if "natural_log_exp_and_others" not in tables:
            return tables
        return {
            name: (funcs if name == "natural_log_exp_and_others" else set())
            for name, funcs in tables.items()
        }

    bacc.get_activation_tables = pinned
    bacc._act_tables_pinned = True


def _build(num_iters: int, general_bias: bool, reps: int = 1):
    import concourse.bass as bass
    import concourse.bacc as bacc
    import concourse.tile as tile
    from concourse import mybir

    _pin_act_table()

    f32 = mybir.dt.float32
    bf16 = mybir.dt.bfloat16
    AF = mybir.ActivationFunctionType
    AX = mybir.AxisListType
    OP = mybir.AluOpType

    nc = bacc.Bacc('TRN2', target_bir_lowering=False, debug=False, enable_asserts=False, num_devices=NCORES)

    # ---------------- dram I/O ----------------
    xdt = mybir.dt.float8e4 if X_FP8 else bf16
    x_d = nc.dram_tensor("x", [BEX, N, FEAT_DIM], xdt, kind="ExternalInput")
    xT_d = nc.dram_tensor("xT", [BEX, FEAT_DIM, N], bf16, kind="ExternalInput")
    slots_d = nc.dram_tensor("slots0", [128, SLOT_DIM], f32, kind="ExternalInput")
    wkv_d = nc.dram_tensor("wkv", [FEAT_DIM, 256], bf16, kind="ExternalInput")
    ckv_d = nc.dram_tensor("ckv", [1, 256], bf16, kind="ExternalInput")
    wq_d = nc.dram_tensor("wq", [SLOT_DIM, SLOT_DIM], bf16, kind="ExternalInput")
    bqs_col_d = nc.dram_tensor("bqs_col", [128, 1], f32, kind="ExternalInput")
    wih_d = nc.dram_tensor("wihT", [SLOT_DIM, 3 * SLOT_DIM], bf16, kind="ExternalInput")
    whh_d = nc.dram_tensor("whhT", [SLOT_DIM, 3 * SLOT_DIM], bf16, kind="ExternalInput")
    bih_d = nc.dram_tensor("bih_row", [1, 3 * SLOT_DIM], bf16, kind="ExternalInput")
    bhh_d = nc.dram_tensor("bhh_row", [1, 3 * SLOT_DIM], bf16, kind="ExternalInput")
    w1_d = nc.dram_tensor("w1", [SLOT_DIM, HID_DIM], bf16, kind="ExternalInput")
    b1c_d = nc.dram_tensor("b1_cols", [128, 4], f32, kind="ExternalInput")
    w2_d = nc.dram_tensor("w2", [HID_DIM, SLOT_DIM], bf16, kind="ExternalInput")
    b2_d = nc.dram_tensor("b2_row", [1, SLOT_DIM], bf16, kind="ExternalInput")
    ones_b_d = nc.dram_tensor("ones_b", [128, 128], bf16, kind="ExternalInput")
    ones_fr_d = nc.dram_tensor("ones_f_row", [1, 128], f32, kind="ExternalInput")
    ident_d = nc.dram_tensor("ident", [128, 128], f32, kind="ExternalInput")
    nident_d = nc.dram_tensor("nident", [128, 128], f32, kind="ExternalInput")
    if general_bias:
        bkv_d = nc.dram_tensor("bkv", [1, 256], f32, kind="ExternalInput")
        bk_colb_d = nc.dram_tensor("bk_col_b", [128, 1], bf16, kind="ExternalInput")
    out_d = nc.dram_tensor("out", [128, SLOT_DIM], f32, kind="ExternalOutput")

    from contextlib import ExitStack

    with tile.TileContext(nc) as tc:
        with ExitStack() as stack:
            pool = lambda *a, **k: stack.enter_context(tc.tile_pool(*a, **k))
            kvp = pool(name="kv", bufs=1)            # resident k/v (16MB)
            cp = pool(name="consts", bufs=1)
            srp = pool(name="statr", bufs=1)         # resident per-ex rstd
            xp = pool(name="xq", bufs=3)
            xtp = pool(name="xtq", bufs=2)
            stp = pool(name="stq", bufs=2)
            nmp = pool(name="nmup", bufs=2)
            atp = pool(name="attnS", bufs=7)         # per-ex attn tiles
            edp = pool(name="eds", bufs=3)
            dnp = pool(name="denb", bufs=4)
            qbp = pool(name="qb", bufs=2)
            gp = pool(name="gru", bufs=1)
            slp = pool(name="slt", bufs=2)
            pa = pool(name="pbig", bufs=2, space="PSUM")     # kps/dps [128,512] f32, 2 banks
            pb = pool(name="pv", bufs=2, space="PSUM")       # v 4-block banks, 2 banks
            pc = pool(name="psc", bufs=2, space="PSUM")      # shared scratch banks, 2 banks
            pd = pool(name="pgate", bufs=1, space="PSUM")    # gi/gh [128,384], 2 banks
            dp = pool(name="dram", bufs=3, space="DRAM")
            p1ps = pa
            pdots = pa
            pgate = pd
            # ---- resident k/v ----
            kT = [kvp.tile([128, N], bf16, tag=f"kT{e}", name=f"kT{e}") for e in range(BEX)]
            vN = [kvp.tile([128, N], bf16, tag=f"v{e}", name=f"v{e}") for e in range(BEX)]
            rstd_nat = [srp.tile([128, NBLK], f32, tag=f"rstd{e}", name=f"rstd{e}") for e in range(BEX)]
            std_b = [srp.tile([128, NBLK], bf16, tag=f"std{e}", name=f"std{e}") for e in range(BEX)]

            # ---- constants ----
            wkv_sb = cp.tile([FEAT_DIM // 4, 4, 256], bf16)  # [128f, fch, 256]
            for j in range(4):
                nc.sync.dma_start(out=wkv_sb[:, j, :], in_=wkv_d[j * 128:(j + 1) * 128, :])
            ckv_sb = cp.tile([1, 256], bf16)
            nc.sync.dma_start(out=ckv_sb, in_=ckv_d[:, :])
            wq_sb = cp.tile([128, 128], bf16)
            nc.sync.dma_start(out=wq_sb, in_=wq_d[:, :])
            bqs_sb = cp.tile([128, 1], f32)
            nc.sync.dma_start(out=bqs_sb, in_=bqs_col_d[:, :])
            wih_sb = cp.tile([128, 384], bf16)
            nc.sync.dma_start(out=wih_sb, in_=wih_d[:, :])
            whh_sb = cp.tile([128, 384], bf16)
            nc.sync.dma_start(out=whh_sb, in_=whh_d[:, :])
            bih_sb = cp.tile([1, 384], bf16)
            nc.sync.dma_start(out=bih_sb, in_=bih_d[:, :])
            bhh_sb = cp.tile([1, 384], bf16)
            nc.sync.dma_start(out=bhh_sb, in_=bhh_d[:, :])
            w1_sb = cp.tile([128, 512], bf16)
            nc.sync.dma_start(out=w1_sb, in_=w1_d[:, :])
            b1c_sb = cp.tile([128, 4], f32)
            nc.sync.dma_start(out=b1c_sb, in_=b1c_d[:, :])
            w2_sb = cp.tile([128, 4, 128], bf16)  # [128h, chunk, 128d]
            for j in range(4):
                nc.sync.dma_start(out=w2_sb[:, j, :], in_=w2_d[j * 128:(j + 1) * 128, :])
            b2_sb = cp.tile([1, 128], bf16)
            nc.sync.dma_start(out=b2_sb, in_=b2_d[:, :])
            ones_b = cp.tile([128, 128], bf16)
            nc.sync.dma_start(out=ones_b, in_=ones_b_d[:, :])
            ones_fr = cp.tile([1, 128], f32)
            nc.sync.dma_start(out=ones_fr, in_=ones_fr_d[:, :])
            ident = cp.tile([128, 128], f32)
            nc.sync.dma_start(out=ident, in_=ident_d[:, :])
            nident = cp.tile([128, 128], f32)
            nc.sync.dma_start(out=nident, in_=nident_d[:, :])
            eps_col = cp.tile([128, 1], f32)
            nc.vector.memset(eps_col, EPS_LN)
            neghalf_col = cp.tile([128, 1], f32)
            nc.vector.memset(neghalf_col, -0.5)
            poshalf_col = cp.tile([128, 1], f32)
            nc.vector.memset(poshalf_col, 0.5)
            negone_col = cp.tile([128, 1], f32)
            nc.vector.memset(negone_col, -1.0)
            negtwo_col = cp.tile([128, 1], f32)
            nc.vector.memset(negtwo_col, -2.0)
            if general_bias:
                bk_colb = cp.tile([128, 1], bf16)
                nc.sync.dma_start(out=bk_colb, in_=bk_colb_d[:, :])
                bv_bc = cp.tile([128, 128], f32)
                nc.sync.dma_start(
                    out=bv_bc,
                    in_=bass.AP(tensor=bkv_d, offset=128, ap=[[0, 128], [1, 128]]),
                )

            def layernorm_T(src, tag):
                """LN over free dim of [128,128] f32 src -> lnT bf16 [128,128]."""
                st = gp.tile([128, 6], f32, tag=f"{tag}_st")
                nc.vector.bn_stats(out=st, in_=src)
                mv = gp.tile([128, 2], f32, tag=f"{tag}_mv")
                nc.vector.bn_aggr(out=mv, in_=st)
                lnv = gp.tile([128, 1], f32, tag=f"{tag}_lnv")
                nc.scalar.activation(lnv, mv[:, 1:2], AF.Ln, bias=eps_col)
                rstd = gp.tile([128, 1], f32, tag=f"{tag}_rstd")
                nc.scalar.activation(rstd, lnv, AF.Exp, scale=neghalf_col)
                nmr = gp.tile([128, 1], f32, tag=f"{tag}_nmr")
                nc.vector.scalar_tensor_tensor(nmr, mv[:, 0:1], -1.0, rstd, OP.mult, OP.mult)
                ln = gp.tile([128, 128], f32, tag=f"{tag}_ln")
                nc.scalar.activation(ln, src, AF.Identity, scale=rstd, bias=nmr)
                sc = pc.tile([128, 512], f32, tag="sc")
                nc.tensor.transpose(sc[:, 0:128], ln, ident)
                lnT = gp.tile([128, 128], bf16, tag=f"{tag}_lnT")
                nc.scalar.activation(lnT, sc[:, 0:128], AF.Copy)
                return lnT, sc

            def qpath(slots):
                lnT, sc = layernorm_T(slots, "q")
                qps = sc[:, 128:256]
                nc.tensor.matmul(qps, wq_sb, lnT)
                qT = qbp.tile([128, 128], bf16, tag="qT")
                nc.scalar.activation(qT, qps, AF.Identity, bias=bqs_sb)
                qbk = None
                if general_bias:
                    qbk_ps = sc[0:1, 256:384]
                    nc.tensor.matmul(qbk_ps, bk_colb, qT)
                    qbk = qbp.tile([1, 128], bf16, tag="qbk")
                    nc.scalar.activation(qbk, qbk_ps, AF.Copy)
                return qT, qbk

            def phase1(e):
                for qd in range(NQ):
                    xq = xp.tile([128, 8, 512], xdt, tag="xq")
                    nc.sync.dma_start(
                        out=xq,
                        in_=bass.AP(tensor=x_d, offset=(e * N + qd * 1024) * FEAT_DIM,
                                    ap=[[FEAT_DIM, 128], [128 * FEAT_DIM, 8], [1, FEAT_DIM]]),
                    )
                    xtq = xtp.tile([128, 4, 1024], bf16, tag="xtq")
                    nc.sync.dma_start(
                        out=xtq,
                        in_=bass.AP(tensor=xT_d, offset=e * FEAT_DIM * N + qd * 1024,
                                    ap=[[N, 128], [128 * N, 4], [1, 1024]]),
                    )
                    st = stp.tile([128, 8, 6], f32, tag="st")
                    mvq = stp.tile([128, 8, 2], f32, tag="mv")
                    for t8 in range(8):
                        if qd < 3 and t8 == 0:
                            # Act-accumulated stats: sum(x^2) and sum(x)
                            sq = stp.tile([128, 512], bf16, tag="ascrap")
                            s2c = stp.tile([128, 1], f32, tag="s2c")
                            nc.scalar.activation(sq, xq[:, t8, :], AF.Square,
                                                 accum_out=s2c)
                            cpx = stp.tile([128, 512], bf16, tag="ascrap")
                            s1c = stp.tile([128, 1], f32, tag="s1c")
                            nc.scalar.activation(cpx, xq[:, t8, :], AF.Copy,
                                                 accum_out=s1c)
                            nc.vector.tensor_scalar_mul(mvq[:, t8, 0:1], s1c,
                                                        1.0 / FEAT_DIM)
                            msq = stp.tile([128, 1], f32, tag="msq")
                            nc.vector.tensor_mul(msq, mvq[:, t8, 0:1], mvq[:, t8, 0:1])
                            nc.vector.scalar_tensor_tensor(
                                mvq[:, t8, 1:2], s2c, 1.0 / FEAT_DIM, msq,
                                OP.mult, OP.subtract)
                        else:
                            nc.vector.bn_stats(out=st[:, t8, :], in_=xq[:, t8, :])
                            nc.vector.bn_aggr(out=mvq[:, t8, :], in_=st[:, t8, :])
                    # rstd = exp(-0.5 * ln(var + eps))  (keeps Act on the exp/ln table)
                    lnv8 = stp.tile([128, 8], f32, tag="lnv8")
                    nc.scalar.activation(lnv8, mvq[:, :, 1], AF.Ln, bias=eps_col)
                    nc.scalar.activation(rstd_nat[e][:, 8 * qd:8 * qd + 8], lnv8,
                                         AF.Exp, scale=neghalf_col)
                    nc.scalar.activation(std_b[e][:, 8 * qd:8 * qd + 8], lnv8,
                                         AF.Exp, scale=poshalf_col)
                    # -mu, transposed then flattened to a [1,1024] row
                    # (matmul lhsT/rhs must sit at base partition 0)
                    sc1 = pc.tile([128, 512], f32, tag="sc")
                    nc.tensor.transpose(sc1[0:8, 0:128], mvq[:, :, 0], nident)
                    nmuT_sb = nmp.tile([8, 128], bf16, tag="nmuT")
                    nc.scalar.activation(nmuT_sb, sc1[0:8, 0:128], AF.Copy)
                    dr = dp.tile([8, 128], bf16, tag="bounce")
                    nc.sync.dma_start(out=dr, in_=nmuT_sb)
                    nmu_row = nmp.tile([1, 1024], bf16, tag="nmurow")
                    nc.sync.dma_start(
                        out=nmu_row,
                        in_=bass.AP(tensor=dr.tensor, offset=dr.offset, ap=[[0, 1], [1, 1024]]),
                    )

                    # ---- kT chunks (2 per quarter), unscaled ----
                    for ci in range(2):
                        c = 2 * qd + ci
                        ps = p1ps.tile([128, 512], f32, tag="big512")
                        for j in range(4):
                            nc.tensor.matmul(ps, wkv_sb[:, j, 0:128],
                                             xtq[:, j, ci * 512:(ci + 1) * 512],
                                             start=(j == 0), stop=False)
                        for m4 in range(4):
                            loc = 4 * ci + m4      # tile index within quarter
                            nc.tensor.matmul(ps[:, m4 * 128:(m4 + 1) * 128],
                                             ckv_sb[:, 0:128],
                                             nmu_row[0:1, loc * 128:(loc + 1) * 128],
                                             start=False, stop=(m4 == 3),
                                             skip_group_check=True)
                        nc.scalar.activation(kT[e][:, c * 512:(c + 1) * 512], ps, AF.Copy)

                    # ---- v blocks (8 per quarter), rstd folded at evacuation ----
                    for tb in range(8):
                        t = 8 * qd + tb
                        if tb % 4 == 0:
                            vbank = pb.tile([128, 4, 128], f32, tag="vps")
                        vps = vbank[:, tb % 4, :]
                        for j in range(4):
                            nc.tensor.matmul(vps, xtq[:, j, tb * 128:(tb + 1) * 128],
                                             wkv_sb[:, j, 128:256],
                                             start=(j == 0), stop=False)
                        nc.tensor.matmul(vps, nmu_row[0:1, tb * 128:(tb + 1) * 128],
                                         ckv_sb[:, 128:256],
                                         start=False, stop=True)
                        if general_bias:
                            # bias must not be rstd-scaled; v stays unscaled here
                            # (rstd folds into attn), so scale bv by std at add
                            vs = stp.tile([128, 128], f32, tag="vtmp")
                            nc.scalar.activation(vs, bv_bc, AF.Copy,
                                                 scale=std_b[e][:, t:t + 1])
                            nc.vector.tensor_add(vps, vps, vs)
                        if tb % 4 == 3:
                            nc.scalar.activation(
                                vN[e][:, (t - 3) * 128:(t + 1) * 128], vbank, AF.Copy)

            attn_sb = [None] * BEX

            def attn_front(e, qT, qbk):
                dps = pdots.tile([128, 512], f32, tag="big512")
                for t in range(NBLK):
                    if general_bias:
                        nc.tensor.matmul(dps[:, t * 16:(t + 1) * 16],
                                         kT[e][:, t * 128:(t + 1) * 128],
                                         qT[:, e * 16:(e + 1) * 16],
                                         start=True, stop=False)
                        nc.tensor.matmul(dps[:, t * 16:(t + 1) * 16],
                                         ones_b[0:1, :], qbk[:, e * 16:(e + 1) * 16],
                                         start=False, stop=True)
                    else:
                        nc.tensor.matmul(dps[:, t * 16:(t + 1) * 16],
                                         kT[e][:, t * 128:(t + 1) * 128],
                                         qT[:, e * 16:(e + 1) * 16])
                # k-side rstd applied on the dots (free-broadcast over s)
                ds = edp.tile([128, 512], f32, tag="ds")
                nc.vector.tensor_mul(
                    ds, dps,
                    bass.AP(tensor=rstd_nat[e].tensor, offset=rstd_nat[e].offset,
                            ap=[rstd_nat[e].ap[0], [1, NBLK], [0, 16]]),
                )
                E = edp.tile([128, 512], bf16, tag="E")
                nc.scalar.activation(E, ds, AF.Exp)
                den = dnp.tile([128, NBLK], bf16, tag="den")
                with nc.allow_low_precision(reason="softmax denominator, 16 terms"):
                    nc.vector.reduce_sum(
                        den, bass.AP(tensor=E.tensor, offset=E.offset,
                                     ap=[E.ap[0], [16, NBLK], [1, 16]]),
                        axis=AX.X,
                    )
                rden = dnp.tile([128, NBLK], f32, tag="rden")
                nc.vector.reciprocal(rden, den)
                rr = dnp.tile([128, NBLK], f32, tag="rr")
                nc.vector.tensor_mul(rr, rden, rstd_nat[e])
                attn = atp.tile([128, 512], bf16, tag="attn")
                nc.gpsimd.tensor_mul(
                    attn, E,
                    bass.AP(tensor=rr.tensor, offset=rr.offset,
                            ap=[rr.ap[0], [1, NBLK], [0, 16]]),
                )
                attn_sb[e] = attn

            def attn_back(e, updT):
                attn = attn_sb[e]
                sc = pc.tile([128, 512], f32, tag="sc")
                ups = sc[:, 0:16]
                zps = sc[0:1, 16:32]
                zbc = sc[:, 32:48]
                for t in range(NBLK):
                    nc.tensor.matmul(ups, vN[e][:, t * 128:(t + 1) * 128],
                                     attn[:, t * 16:(t + 1) * 16],
                                     start=(t == 0), stop=(t == NBLK - 1))
                for t in range(NBLK):
                    nc.tensor.matmul(zps, std_b[e][:, t:t + 1],
                                     attn[:, t * 16:(t + 1) * 16],
                                     start=(t == 0), stop=(t == NBLK - 1))
                rz = dnp.tile([1, 16], f32, tag="rz")
                nc.vector.reciprocal(rz, zps)
                nc.tensor.matmul(zbc, ones_fr, rz)
                zbs = dnp.tile([128, 16], f32, tag="zbs")
                nc.scalar.activation(zbs, zbc, AF.Copy)
                nc.vector.tensor_mul(updT[:, e * 16:(e + 1) * 16], ups, zbs)

            def sigmoid_via_exp(out, in_ap, width, tag):
                e1 = gp.tile([128, width], f32, tag=f"{tag}_e")
                nc.scalar.activation(e1, in_ap, AF.Exp, scale=negone_col)
                p1 = gp.tile([128, width], f32, tag=f"{tag}_p")
                nc.vector.tensor_scalar_add(p1, e1, 1.0)
                nc.vector.reciprocal(out, p1)

            def grumlp(slots, updT):
                gips = pgate.tile([128, 384], f32, tag="gi")
                nc.tensor.matmul(gips, updT, wih_sb, start=True, stop=False)
                nc.tensor.matmul(gips, ones_b[0:1, :], bih_sb, start=False, stop=True)
                scg = pc.tile([128, 512], f32, tag="sc")
                nc.tensor.transpose(scg[:, 0:128], slots, ident)
                slotsT = gp.tile([128, 128], bf16, tag="slotsT")
                nc.scalar.activation(slotsT, scg[:, 0:128], AF.Copy)
                ghps = pgate.tile([128, 384], f32, tag="gh")
                nc.tensor.matmul(ghps, slotsT, whh_sb, start=True, stop=False)
                nc.tensor.matmul(ghps, ones_b[0:1, :], bhh_sb, start=False, stop=True)
                gh_sb = gp.tile([128, 384], f32, tag="ghsb")
                nc.scalar.activation(gh_sb, ghps, AF.Copy)
                rzin = gp.tile([128, 256], f32, tag="rzin")
                nc.vector.tensor_add(rzin, gips[:, 0:256], gh_sb[:, 0:256])
                rzg = gp.tile([128, 256], f32, tag="rzg")
                sigmoid_via_exp(rzg, rzin, 256, "sg")
                hnr = gp.tile([128, 128], f32, tag="hnr")
                nc.vector.tensor_mul(hnr, rzg[:, 0:128], gh_sb[:, 256:384])
                nin = gp.tile([128, 128], f32, tag="nin")
                nc.vector.tensor_add(nin, gips[:, 256:384], hnr)
                # tanh(x) = 2/(1+exp(-2x)) - 1
                e2 = gp.tile([128, 128], f32, tag="th_e")
                nc.scalar.activation(e2, nin, AF.Exp, scale=negtwo_col)
                p2 = gp.tile([128, 128], f32, tag="th_p")
                nc.vector.tensor_scalar_add(p2, e2, 1.0)
                r2 = gp.tile([128, 128], f32, tag="th_r")
                nc.vector.reciprocal(r2, p2)
                ng = gp.tile([128, 128], f32, tag="ng")
                nc.vector.tensor_scalar(ng, r2, 2.0, -1.0, OP.mult, OP.add)
                hmn = gp.tile([128, 128], f32, tag="hmn")
                nc.vector.tensor_sub(hmn, slots, ng)
                zh = gp.tile([128, 128], f32, tag="zh")
                nc.vector.tensor_mul(zh, rzg[:, 128:256], hmn)
                hgru = gp.tile([128, 128], f32, tag="hgru")
                nc.vector.tensor_add(hgru, ng, zh)

                lnmT, scm = layernorm_T(hgru, "m")
                h1r = gp.tile([128, 4, 128], bf16, tag="h1r")
                sch = pc.tile([128, 512], f32, tag="sc")
                for j in range(4):
                    hp = sch[:, j * 128:(j + 1) * 128]
                    nc.tensor.matmul(hp, w1_sb[:, j * 128:(j + 1) * 128], lnmT)
                    nc.scalar.activation(h1r[:, j, :], hp, AF.Relu, bias=b1c_sb[:, j:j + 1])
                h2ps = scm[:, 128:256]
                for j in range(4):
                    nc.tensor.matmul(h2ps, h1r[:, j, :], w2_sb[:, j, :],
                                     start=(j == 0), stop=False)
                nc.tensor.matmul(h2ps, ones_b[0:1, :], b2_sb, start=False, stop=True)
                new_slots = slp.tile([128, 128], f32, tag="slots")
                nc.vector.tensor_add(new_slots, h2ps, hgru)
                return new_slots

            for _rep in range(reps):
                slots = slp.tile([128, 128], f32, tag="slots")
                nc.sync.dma_start(out=slots, in_=slots_d[:, :])

                # iter-1 q depends only on initial slots: compute up front
                qT1, qbk1 = qpath(slots)
                updT1 = qbp.tile([128, 128], bf16, tag="updT")
                for e in range(BEX):
                    if e >= 1:
                        attn_front(e - 1, qT1, qbk1)
                    phase1(e)
                    if e >= 2:
                        attn_back(e - 2, updT1)
                attn_front(BEX - 1, qT1, qbk1)
                attn_back(BEX - 2, updT1)
                attn_back(BEX - 1, updT1)
                slots = grumlp(slots, updT1)

                for _it in range(1, num_iters):
                    qT, qbk = qpath(slots)
                    updT = qbp.tile([128, 128], bf16, tag="updT")
                    for e in range(BEX):
                        attn_front(e, qT, qbk)
                    for e in range(BEX):
                        attn_back(e, updT)
                    slots = grumlp(slots, updT)

                nc.sync.dma_start(out=out_d[:, :], in_=slots)

    nc.finalize()
    return nc


def _prep_host(inputs):
    f = np.float32
    bf = ml_dtypes.bfloat16
    g_in = inputs["ln_in_g"].astype(f)
    b_in = inputs["ln_in_b"].astype(f)
    Wk = inputs["Wk"].astype(f)
    Wv = inputs["Wv"].astype(f)
    Wkp = g_in[:, None] * Wk
    Wvp = g_in[:, None] * Wv
    wkv = np.concatenate([Wkp, Wvp], axis=1)                      # [512, 256]
    ckv = wkv.sum(axis=0, keepdims=True)                          # [1, 256]
    bk = b_in @ Wk + inputs["bk"].astype(f)
    bv = b_in @ Wv + inputs["bv"].astype(f)
    bkv = np.concatenate([bk, bv])[None, :]                       # [1, 256]
    g_s = inputs["ln_slot_g"].astype(f)
    b_s = inputs["ln_slot_b"].astype(f)
    Wq = inputs["Wq"].astype(f)
    wqp = g_s[:, None] * Wq
    bqs = (b_s @ Wq + inputs["bq"].astype(f)) * np.float32(SCALE)
    g_m = inputs["ln_mlp_g"].astype(f)
    b_m = inputs["ln_mlp_b"].astype(f)
    W1 = inputs["W1"].astype(f)
    w1p = g_m[:, None] * W1
    b1p = b_m @ W1 + inputs["b1"].astype(f)                       # [512]
    consts = dict(
        wkv=wkv.astype(bf),
        ckv=ckv.astype(bf),
        wq=(wqp * np.float32(SCALE)).astype(bf),
        bqs_col=bqs[:, None].astype(f),
        wihT=np.ascontiguousarray(inputs["W_ih"].astype(f).T).astype(bf),
        whhT=np.ascontiguousarray(inputs["W_hh"].astype(f).T).astype(bf),
        bih_row=inputs["b_ih"].astype(f)[None, :].astype(bf),
        bhh_row=inputs["b_hh"].astype(f)[None, :].astype(bf),
        w1=w1p.astype(bf),
        b1_cols=np.ascontiguousarray(b1p.reshape(4, 128).T).astype(f),
        w2=inputs["W2"].astype(f).astype(bf),
        b2_row=inputs["b2"].astype(f)[None, :].astype(bf),
        ones_b=np.ones((128, 128), bf),
        ones_f_row=np.ones((1, 128), f),
        ident=np.eye(128, dtype=f),
        nident=(-np.eye(128)).astype(f),
    )
    general_bias = not (
        np.all(b_in == 0) and np.all(inputs["bk"] == 0) and np.all(inputs["bv"] == 0)
    )
    if general_bias:
        consts["bkv"] = bkv.astype(f)
        consts["bk_col_b"] = bk[:, None].astype(bf)
    return consts, general_bias


LAST_RESULT = None


def kernel(**inputs) -> np.ndarray:
    import os
    from concourse.bass_utils import run_bass_kernel_spmd

    is_first = int(np.asarray(inputs["is_first"]))
    num_iters = 3 if is_first else 2
    consts, general_bias = _prep_host(inputs)

    key = (num_iters, general_bias)
    if key not in _CACHE:
        _CACHE[key] = _build(num_iters, general_bias)
    nc = _CACHE[key]

    bf = ml_dtypes.bfloat16
    x = inputs["image_features"].astype(np.float32)
    xb = x.astype(ml_dtypes.float8_e4m3 if X_FP8 else bf)         # [64, 4096, 512]
    xTb = np.ascontiguousarray(x.transpose(0, 2, 1)).astype(bf)   # [64, 512, 4096]
    slots = inputs["slots"].astype(np.float32)                    # [64, 16, 128]

    in_maps = []
    for c in range(NCORES):
        sl = slice(c * BEX, (c + 1) * BEX)
        m = dict(consts)
        m["x"] = xb[sl]
        m["xT"] = xTb[sl]
        m["slots0"] = slots[sl].reshape(128, SLOT_DIM)
        in_maps.append(m)

    trace = bool(int(os.environ.get("KERNEL_TRACE", "0")))
    res = run_bass_kernel_spmd(nc, in_maps, list(range(NCORES)), trace=trace)
    global LAST_RESULT
    LAST_RESULT = res
    out = np.stack([res.results[c]["out"] for c in range(NCORES)])  # [8, 128, 128]
    return out.reshape(B, NUM_SLOTS, SLOT_DIM)


if __name__ == "__main__":
    import reference
    inp = reference.setup_inputs()
    inp = {k: np.asarray(v) for k, v in inp.items()}
    got = kernel(**inp)
    exp = np.asarray(reference.reference(**reference.setup_inputs()))
    err = np.linalg.norm(got - exp) / np.linalg.norm(exp)
    print("Relative error:", err)
